# revision 22
# baseline (speedup 1.0000x reference)
"""GNN ensemble MoE-routing kernel for Trainium2 (8 NeuronCores).

Reference computes all 8 expert MLPs for every sample then selects one
(8x wasted FLOPs). This kernel routes on the host instead: samples are
gathered per expert, and core c runs ONLY expert c's MLP over the
samples routed to it (expert-parallel sharding).

Math folding (exact):
  lat = eps*sigma_c + mu_c  =>  lat @ W1_c = eps @ (sigma_c*W1_c) + mu_c@W1_c
so the device computes  sigmoid(eps @ W1p + b1p) @ W2 + b2  with
  W1p = sigma_c * W1_c,  b1p = b1_c + mu_c @ W1_c  (folded on host).

Device layout: features on SBUF partitions, samples on the free axis.
  epsT [512, K]  ->  H^T [1024, K]  ->  Y^T [512, K]
All matmul operands are bf16 (1 cycle/row at any free-dim size, vs
fp32r's >=256 restriction; also halves DMA bytes and LDWEIGHTS time).
Measured bf16 end-to-end rms rel err ~3e-3, well under the 2e-2 gate.

DMA descriptor writes cost ~0.6us each (serialized per HWDGE ring), so
transfers are packed: the 4 partition-block columns of each input chunk
ship as ONE descriptor (host pre-packs them side-by-side per chunk),
each chunk's 4 output blocks ship as one descriptor, w2 is one packed
[128, 4096] tile (2 transfers), b1+b2 share one [128, 12] f32 tile.
Weights go on the Sync HWDGE ring, activations on the Scalar ring, so
the first matmul's two dependencies land in parallel; later chunks'
loads are deferred into the compute stream so the bandwidth-limited
early drain (~220 GB/s/core, chip-HBM-bound with 8 cores loading
weights at once) serves the weights first. Dummy matmuls on a memset
tile warm the PE out of its cold p-state while the first loads are in
flight.

Chunks are [512, ..., rem-256, 256]: big chunks amortize LDWEIGHTS, the
small last chunk shortens the post-matmul tail (bias-add + final
store, split in two so the first half stores early). k_cap rounds the
max expert count to 8, minimizing padded columns.
"""

from contextlib import ExitStack

import numpy as np
import ml_dtypes

import concourse.bass as bass
import concourse.tile as tile
from concourse import bacc, mybir
from concourse.bass_utils import run_bass_kernel_spmd

NB_COMP = 8
LAT_DIM = 512
NB_NEUR = 1024
OUT_DIM = 512
N_CORES = 8
KC1 = LAT_DIM // 128   # 4 input row-blocks
MC1 = NB_NEUR // 128   # 8 hidden row-blocks
MC2 = OUT_DIM // 128   # 4 output row-blocks

F32 = mybir.dt.float32
BF16 = mybir.dt.bfloat16
NP_BF16 = ml_dtypes.bfloat16
SIG = mybir.ActivationFunctionType.Sigmoid

_program_cache = {}


def _make_chunks(k_cap):
    # 512-wide chunks; the remainder splits so the LAST chunk is small
    # (<=256): it defines the post-matmul tail (bias-add + final store).
    chunks = []
    n0 = 0
    while k_cap - n0 > 576:
        chunks.append((n0, 512))
        n0 += 512
    rem = k_cap - n0
    if rem > 256:
        chunks.append((n0, rem - 256))
        n0 += rem - 256
        rem = 256
    if rem:
        chunks.append((n0, rem))
    return chunks


def _build_program(k_cap):
    """One-expert MLP over k_cap samples; same program runs SPMD on all 8 cores."""
    chunks = _make_chunks(k_cap)

    nc = bacc.Bacc(
        "TRN2",
        target_bir_lowering=False,
        debug=False,
        enable_asserts=False,
        num_devices=N_CORES,
    )
    # chunk-packed layouts: chunk ci at columns [KC1*n0, KC1*(n0+ns)) with
    # its KC1 (resp. MC2) partition-blocks side by side.
    epsC = nc.dram_tensor("epsC", [128, KC1 * k_cap], BF16, kind="ExternalInput").ap()
    w1 = nc.dram_tensor("w1", [LAT_DIM, NB_NEUR], BF16, kind="ExternalInput").ap()
    w2 = nc.dram_tensor("w2", [128, MC1 * OUT_DIM], BF16, kind="ExternalInput").ap()
    bb = nc.dram_tensor("bb", [128, MC1 + MC2], F32, kind="ExternalInput").ap()
    yC = nc.dram_tensor("yC", [128, MC2 * k_cap], BF16, kind="ExternalOutput").ap()

    with tile.TileContext(nc) as tc, ExitStack() as ctx:
        wpool = ctx.enter_context(tc.tile_pool(name="weights", bufs=1))
        xpool = ctx.enter_context(tc.tile_pool(name="x", bufs=len(chunks)))
        hpool = ctx.enter_context(tc.tile_pool(name="h", bufs=2))
        ypool = ctx.enter_context(tc.tile_pool(name="y", bufs=2))
        # mm1 keeps 8 accumulators live (kc-outer order), mm2 4, cycling
        # through the same 8 PSUM banks.
        pspool = ctx.enter_context(tc.tile_pool(name="ps", bufs=8, space="PSUM"))

        # Warm-up: the PE runs at a reduced p-state until ~3us of
        # continuous execution. Run dummy matmuls on a memset tile while
        # the first weight/input DMAs are in flight so real matmuls start
        # at full clock.
        warm = wpool.tile([128, 640], BF16, tag="warm")
        nc.gpsimd.memset(warm[:], 0)
        ps_w = pspool.tile([128, 512], F32, tag="ps", name="ps_warm")
        for _ in range(4):
            nc.tensor.matmul(
                ps_w[:], warm[:, 0:128], warm[:, 128:640], start=True, stop=True
            )

        # Two physical HWDGE rings exist (Sync and Scalar), FIFO per issuing
        # engine, ~0.6us per descriptor write. Split the input stream across
        # them: weights on Sync, activations (and the tiny bias pack) on
        # Scalar, so the first matmul's dependencies (w1_0 on one ring, the
        # first x block on the other) land in parallel. w2 is split in two
        # so mm2's kc<4 half can arrive before its first use.
        w1t = []
        t = wpool.tile([128, NB_NEUR], BF16, tag="w1_0", name="w1_0")
        # only the mc=0 slice of w1_0 gates the first real matmul's weight
        # load; ship it first as its own tiny transfer.
        nc.sync.dma_start(t[:, 0:128], w1[0:128, 0:128])
        nc.sync.dma_start(t[:, 128:], w1[0:128, 128:])
        w1t.append(t)
        n0_0, ns_0 = chunks[0]
        # split the first chunk's load: only the kc=0 block gates the first
        # real matmul, the rest follows while kc=0 computes.
        x0 = xpool.tile([128, KC1 * ns_0], BF16, tag="x", name="x0")
        nc.scalar.dma_start(x0[:, :ns_0], epsC[:, 0:ns_0])
        for kc in range(1, KC1):
            t = wpool.tile([128, NB_NEUR], BF16, tag=f"w1_{kc}", name=f"w1_{kc}")
            nc.sync.dma_start(t[:], w1[kc * 128 : (kc + 1) * 128, :])
            w1t.append(t)
        nc.scalar.dma_start(x0[:, ns_0:], epsC[:, ns_0 : KC1 * ns_0])
        bbt = wpool.tile([128, MC1 + MC2], F32, tag="bb")
        nc.scalar.dma_start(bbt[:], bb[:])
        w2t = wpool.tile([128, MC1 * OUT_DIM], BF16, tag="w2")
        half = MC1 * OUT_DIM // 2
        nc.sync.dma_start(w2t[:, :half], w2[:, :half])
        nc.sync.dma_start(w2t[:, half:], w2[:, half:])
        # Software pipeline: mm1(ci+1) runs BEFORE mm2(ci), so every w2 /
        # x-chunk deadline moves one whole mm1 stage (~7us) later. The head
        # DMA stream is bandwidth-limited (~220 GB/s/core, chip-HBM-bound);
        # without the lookahead, chunk0's mm2 start sits exactly at w2's
        # arrival, so any per-core DMA variance stalls the PE. x1 loads
        # upfront (its deadline is mm1(1), right after mm1(0)); later
        # chunks' loads are emitted at the previous chunk's first
        # activation (Scalar program order).
        xts = [x0]
        for n0, ns in chunks[1:]:
            xts.append(xpool.tile([128, KC1 * ns], BF16, tag="x", name=f"x_{n0}"))

        def load_x(ci):
            n0, ns = chunks[ci]
            nc.scalar.dma_start(xts[ci][:], epsC[:, KC1 * n0 : KC1 * (n0 + ns)])

        if len(chunks) > 1:
            load_x(1)

        hts = {}

        def emit_mm1(ci):
            n0, ns = chunks[ci]
            xt = xts[ci]
            ht = []
            ps1 = [
                pspool.tile([128, ns], F32, tag="ps", name=f"ps1_{ci}_{i}")
                for i in range(MC1)
            ]
            for kc in range(KC1):
                for mc in range(MC1):
                    nc.tensor.matmul(
                        ps1[mc][:],
                        w1t[kc][:, mc * 128 : (mc + 1) * 128],
                        xt[:, kc * ns : (kc + 1) * ns],
                        start=(kc == 0),
                        stop=(kc == KC1 - 1),
                    )
                    if kc == KC1 - 1:
                        h = hpool.tile([128, ns], BF16, tag=f"h{mc}")
                        nc.scalar.activation(
                            h[:], ps1[mc][:], SIG, bias=bbt[:, mc : mc + 1]
                        )
                        ht.append(h)
                        if mc == 0 and ci + 2 < len(chunks):
                            load_x(ci + 2)
            hts[ci] = ht

        def emit_mm2(ci):
            n0, ns = chunks[ci]
            ht = hts.pop(ci)
            yt = ypool.tile([128, MC2 * ns], BF16, tag="y", name=f"y_{ci}")
            ps2 = [
                pspool.tile([128, ns], F32, tag="ps", name=f"ps2_{ci}_{i}")
                for i in range(MC2)
            ]
            for kc in range(MC1):
                for oc in range(MC2):
                    nc.tensor.matmul(
                        ps2[oc][:],
                        w2t[:, kc * OUT_DIM + oc * 128 : kc * OUT_DIM + (oc + 1) * 128],
                        ht[kc][:],
                        start=(kc == 0),
                        stop=(kc == MC1 - 1),
                    )
                    if kc == MC1 - 1:
                        # split the bias-adds across DVE and ScalarE (GpSimd
                        # cannot read PSUM) so the last chunk's tail is two
                        # parallel pairs, not four serialized ops.
                        if oc % 2 == 0:
                            nc.vector.tensor_scalar_add(
                                yt[:, oc * ns : (oc + 1) * ns],
                                ps2[oc][:],
                                bbt[:, MC1 + oc : MC1 + oc + 1],
                            )
                        else:
                            nc.scalar.activation(
                                yt[:, oc * ns : (oc + 1) * ns],
                                ps2[oc][:],
                                mybir.ActivationFunctionType.Identity,
                                bias=bbt[:, MC1 + oc : MC1 + oc + 1],
                            )
            if ci == len(chunks) - 1:
                # last chunk is the exec-time tail: store the first half as
                # soon as its bias-adds land instead of waiting for all 4.
                nc.sync.dma_start(
                    yC[:, MC2 * n0 : MC2 * n0 + 2 * ns], yt[:, : 2 * ns]
                )
                nc.sync.dma_start(
                    yC[:, MC2 * n0 + 2 * ns : MC2 * (n0 + ns)], yt[:, 2 * ns :]
                )
            else:
                nc.sync.dma_start(yC[:, MC2 * n0 : MC2 * (n0 + ns)], yt[:])

        emit_mm1(0)
        for ci in range(len(chunks)):
            if ci + 1 < len(chunks):
                emit_mm1(ci + 1)
            emit_mm2(ci)

    nc.compile()
    return nc


def get_program(k_cap):
    if k_cap not in _program_cache:
        _program_cache[k_cap] = _build_program(k_cap)
    return _program_cache[k_cap]


def _softplus(x):
    x = x.astype(np.float64)
    return (np.maximum(x, 0.0) + np.log1p(np.exp(-np.abs(x)))).astype(np.float32)


def _pack_chunks(arrT, chunks, nblk):
    """[nblk*128, k_cap] -> [128, nblk*k_cap] chunk-packed."""
    k_cap = arrT.shape[1]
    out = np.empty((128, nblk * k_cap), dtype=arrT.dtype)
    for n0, ns in chunks:
        out[:, nblk * n0 : nblk * (n0 + ns)] = (
            arrT[:, n0 : n0 + ns].reshape(nblk, 128, ns).transpose(1, 0, 2).reshape(128, nblk * ns)
        )
    return out


def _unpack_chunks(arrC, chunks, nblk):
    """[128, nblk*k_cap] chunk-packed -> [nblk*128, k_cap]."""
    k_cap = arrC.shape[1] // nblk
    out = np.empty((nblk * 128, k_cap), dtype=arrC.dtype)
    for n0, ns in chunks:
        out[:, n0 : n0 + ns] = (
            arrC[:, nblk * n0 : nblk * (n0 + ns)].reshape(128, nblk, ns).transpose(1, 0, 2).reshape(nblk * 128, ns)
        )
    return out


def kernel(epsilon, comp_idx, mu, rho, W1, b1, W2, b2, _trace=False):
    epsilon = np.asarray(epsilon, dtype=np.float32)
    comp_idx = np.asarray(comp_idx, dtype=np.int32)
    mu = np.asarray(mu, dtype=np.float32)
    rho = np.asarray(rho, dtype=np.float32)
    W1 = np.asarray(W1, dtype=np.float32)
    b1 = np.asarray(b1, dtype=np.float32)
    W2 = np.asarray(W2, dtype=np.float32)
    b2 = np.asarray(b2, dtype=np.float32)

    n = epsilon.shape[0]
    sigma = _softplus(rho)  # [C]

    sels = [np.nonzero(comp_idx == c)[0] for c in range(NB_COMP)]
    counts = [len(s) for s in sels]
    k_cap = max(64, -(-max(counts) // 8) * 8)

    nc = get_program(k_cap)
    chunks = _make_chunks(k_cap)

    in_maps = []
    for c in range(NB_COMP):
        sel = sels[c]
        epsT = np.zeros((LAT_DIM, k_cap), dtype=NP_BF16)
        if len(sel):
            epsT[:, : len(sel)] = epsilon[sel].T
        w1p = (W1[c] * sigma[c]).astype(NP_BF16)
        b1p = (
            b1[c].astype(np.float64) + mu[c].astype(np.float64) @ W1[c].astype(np.float64)
        ).astype(np.float32)
        bb = np.empty((128, MC1 + MC2), dtype=np.float32)
        bb[:, :MC1] = b1p.reshape(MC1, 128).T
        bb[:, MC1:] = b2[c].reshape(MC2, 128).T
        w2p = np.ascontiguousarray(
            W2[c].astype(NP_BF16).reshape(MC1, 128, OUT_DIM).transpose(1, 0, 2).reshape(128, MC1 * OUT_DIM)
        )
        in_maps.append(
            {
                "epsC": _pack_chunks(epsT, chunks, KC1),
                "w1": np.ascontiguousarray(w1p),
                "w2": w2p,
                "bb": bb,
            }
        )

    res = run_bass_kernel_spmd(
        nc,
        in_maps,
        core_ids=list(range(N_CORES)),
        trace=_trace,
        trace_cores=list(range(N_CORES)) if _trace else None,
    )

    out = np.zeros((n, OUT_DIM), dtype=np.float32)
    for c in range(NB_COMP):
        sel = sels[c]
        if len(sel):
            yT = _unpack_chunks(res.results[c]["yC"], chunks, MC2)
            out[sel] = yT[:, : len(sel)].T.astype(np.float32)
    if _trace:
        return out, res
    return out


# revision 23
# speedup vs baseline: 1.0368x; 1.0368x over previous
"""GNN ensemble MoE-routing kernel for Trainium2 (8 NeuronCores).

Reference computes all 8 expert MLPs for every sample then selects one
(8x wasted FLOPs). This kernel routes on the host instead: samples are
gathered per expert, and core c runs ONLY expert c's MLP over the
samples routed to it (expert-parallel sharding).

Math folding (exact):
  lat = eps*sigma_c + mu_c  =>  lat @ W1_c = eps @ (sigma_c*W1_c) + mu_c@W1_c
so the device computes  sigmoid(eps @ W1p + b1p) @ W2 + b2  with
  W1p = sigma_c * W1_c,  b1p = b1_c + mu_c @ W1_c  (folded on host).

Device layout: features on SBUF partitions, samples on the free axis.
  epsT [512, K]  ->  H^T [1024, K]  ->  Y^T [512, K]
All matmul operands are bf16 (1 cycle/row at any free-dim size, vs
fp32r's >=256 restriction; also halves DMA bytes and LDWEIGHTS time).
Measured bf16 end-to-end rms rel err ~3e-3, well under the 2e-2 gate.

DMA descriptor writes cost ~0.6us each (serialized per HWDGE ring), so
transfers are packed: the 4 partition-block columns of each input chunk
ship as ONE descriptor (host pre-packs them side-by-side per chunk),
each chunk's 4 output blocks ship as one descriptor, w2 is one packed
[128, 4096] tile (2 transfers), b1+b2 share one [128, 12] f32 tile.
Weights go on the Sync HWDGE ring, activations on the Scalar ring, so
the first matmul's two dependencies land in parallel; later chunks'
loads are deferred into the compute stream so the bandwidth-limited
early drain (~220 GB/s/core, chip-HBM-bound with 8 cores loading
weights at once) serves the weights first. Dummy matmuls on a memset
tile warm the PE out of its cold p-state while the first loads are in
flight.

Chunks are [512, ..., rem-256, 256]: big chunks amortize LDWEIGHTS, the
small last chunk shortens the post-matmul tail (bias-add + final
store, split in two so the first half stores early). k_cap rounds the
max expert count to 8, minimizing padded columns.
"""

from contextlib import ExitStack

import numpy as np
import ml_dtypes

import concourse.bass as bass
import concourse.tile as tile
from concourse import bacc, mybir
from concourse.bass_utils import run_bass_kernel_spmd

NB_COMP = 8
LAT_DIM = 512
NB_NEUR = 1024
OUT_DIM = 512
N_CORES = 8
KC1 = LAT_DIM // 128   # 4 input row-blocks
MC1 = NB_NEUR // 128   # 8 hidden row-blocks
MC2 = OUT_DIM // 128   # 4 output row-blocks

F32 = mybir.dt.float32
BF16 = mybir.dt.bfloat16
NP_BF16 = ml_dtypes.bfloat16
SIG = mybir.ActivationFunctionType.Sigmoid

_program_cache = {}


def _make_chunks(k_cap):
    # 512-wide chunks; the remainder splits so the LAST chunk is small
    # (<=256): it defines the post-matmul tail (bias-add + final store).
    chunks = []
    n0 = 0
    while k_cap - n0 > 576:
        chunks.append((n0, 512))
        n0 += 512
    rem = k_cap - n0
    if rem > 256:
        chunks.append((n0, rem - 256))
        n0 += rem - 256
        rem = 256
    if rem:
        chunks.append((n0, rem))
    return chunks


def _build_program(k_cap):
    """One-expert MLP over k_cap samples; same program runs SPMD on all 8 cores."""
    chunks = _make_chunks(k_cap)

    nc = bacc.Bacc(
        "TRN2",
        target_bir_lowering=False,
        debug=False,
        enable_asserts=False,
        num_devices=N_CORES,
    )
    # chunk-packed layouts: chunk ci at columns [KC1*n0, KC1*(n0+ns)) with
    # its KC1 (resp. MC2) partition-blocks side by side.
    epsC = nc.dram_tensor("epsC", [128, KC1 * k_cap], BF16, kind="ExternalInput").ap()
    w1 = nc.dram_tensor("w1", [LAT_DIM, NB_NEUR], BF16, kind="ExternalInput").ap()
    w2 = nc.dram_tensor("w2", [128, MC1 * OUT_DIM], BF16, kind="ExternalInput").ap()
    bb = nc.dram_tensor("bb", [128, MC1 + MC2], F32, kind="ExternalInput").ap()
    yC = nc.dram_tensor("yC", [128, MC2 * k_cap], BF16, kind="ExternalOutput").ap()

    with tile.TileContext(nc) as tc, ExitStack() as ctx:
        wpool = ctx.enter_context(tc.tile_pool(name="weights", bufs=1))
        xpool = ctx.enter_context(tc.tile_pool(name="x", bufs=len(chunks)))
        hpool = ctx.enter_context(tc.tile_pool(name="h", bufs=2))
        ypool = ctx.enter_context(tc.tile_pool(name="y", bufs=2))
        # mm1 keeps 8 accumulators live (kc-outer order), mm2 4, cycling
        # through the same 8 PSUM banks.
        pspool = ctx.enter_context(tc.tile_pool(name="ps", bufs=8, space="PSUM"))

        # Warm-up: the PE runs at a reduced p-state until ~3us of
        # continuous execution. Run dummy matmuls on a memset tile while
        # the first weight/input DMAs are in flight so real matmuls start
        # at full clock.
        warm = wpool.tile([128, 640], BF16, tag="warm")
        nc.gpsimd.memset(warm[:], 0)
        ps_w = pspool.tile([128, 512], F32, tag="ps", name="ps_warm")
        for _ in range(6):
            nc.tensor.matmul(
                ps_w[:], warm[:, 0:128], warm[:, 128:640], start=True, stop=True
            )

        # Two physical HWDGE rings exist (Sync and Scalar), FIFO per issuing
        # engine, ~0.6us per descriptor write. Split the input stream across
        # them: weights on Sync, activations (and the tiny bias pack) on
        # Scalar, so the first matmul's dependencies (w1_0 on one ring, the
        # first x block on the other) land in parallel. w2 is split in two
        # so mm2's kc<4 half can arrive before its first use.
        w1t = []
        t = wpool.tile([128, NB_NEUR], BF16, tag="w1_0", name="w1_0")
        # only the mc=0 slice of w1_0 gates the first real matmul's weight
        # load; ship it first as its own tiny transfer.
        nc.sync.dma_start(t[:, 0:128], w1[0:128, 0:128])
        nc.sync.dma_start(t[:, 128:], w1[0:128, 128:])
        w1t.append(t)
        n0_0, ns_0 = chunks[0]
        # split the first chunk's load: only the kc=0 block gates the first
        # real matmul, the rest follows while kc=0 computes.
        x0 = xpool.tile([128, KC1 * ns_0], BF16, tag="x", name="x0")
        nc.scalar.dma_start(x0[:, :ns_0], epsC[:, 0:ns_0])
        for kc in range(1, KC1):
            t = wpool.tile([128, NB_NEUR], BF16, tag=f"w1_{kc}", name=f"w1_{kc}")
            nc.sync.dma_start(t[:], w1[kc * 128 : (kc + 1) * 128, :])
            w1t.append(t)
        nc.scalar.dma_start(x0[:, ns_0:], epsC[:, ns_0 : KC1 * ns_0])
        bbt = wpool.tile([128, MC1 + MC2], F32, tag="bb")
        nc.scalar.dma_start(bbt[:], bb[:])
        w2t = wpool.tile([128, MC1 * OUT_DIM], BF16, tag="w2")
        half = MC1 * OUT_DIM // 2
        nc.sync.dma_start(w2t[:, :half], w2[:, :half])
        nc.sync.dma_start(w2t[:, half:], w2[:, half:])
        # Software pipeline: mm1(ci+1) runs BEFORE mm2(ci), so every w2 /
        # x-chunk deadline moves one whole mm1 stage (~7us) later. The head
        # DMA stream is bandwidth-limited (~220 GB/s/core, chip-HBM-bound);
        # without the lookahead, chunk0's mm2 start sits exactly at w2's
        # arrival, so any per-core DMA variance stalls the PE. x1 loads
        # upfront (its deadline is mm1(1), right after mm1(0)); later
        # chunks' loads are emitted at the previous chunk's first
        # activation (Scalar program order).
        xts = [x0]
        for n0, ns in chunks[1:]:
            xts.append(xpool.tile([128, KC1 * ns], BF16, tag="x", name=f"x_{n0}"))

        def load_x(ci):
            n0, ns = chunks[ci]
            nc.scalar.dma_start(xts[ci][:], epsC[:, KC1 * n0 : KC1 * (n0 + ns)])

        if len(chunks) > 1:
            load_x(1)

        hts = {}

        def emit_mm1(ci):
            n0, ns = chunks[ci]
            xt = xts[ci]
            ht = []
            ps1 = [
                pspool.tile([128, ns], F32, tag="ps", name=f"ps1_{ci}_{i}")
                for i in range(MC1)
            ]
            for kc in range(KC1):
                for mc in range(MC1):
                    nc.tensor.matmul(
                        ps1[mc][:],
                        w1t[kc][:, mc * 128 : (mc + 1) * 128],
                        xt[:, kc * ns : (kc + 1) * ns],
                        start=(kc == 0),
                        stop=(kc == KC1 - 1),
                    )
                    if kc == KC1 - 1:
                        h = hpool.tile([128, ns], BF16, tag=f"h{mc}")
                        nc.scalar.activation(
                            h[:], ps1[mc][:], SIG, bias=bbt[:, mc : mc + 1]
                        )
                        ht.append(h)
                        if mc == 0 and ci + 2 < len(chunks):
                            load_x(ci + 2)
            hts[ci] = ht

        def emit_mm2(ci):
            n0, ns = chunks[ci]
            ht = hts.pop(ci)
            yt = ypool.tile([128, MC2 * ns], BF16, tag="y", name=f"y_{ci}")
            ps2 = [
                pspool.tile([128, ns], F32, tag="ps", name=f"ps2_{ci}_{i}")
                for i in range(MC2)
            ]
            for kc in range(MC1):
                for oc in range(MC2):
                    nc.tensor.matmul(
                        ps2[oc][:],
                        w2t[:, kc * OUT_DIM + oc * 128 : kc * OUT_DIM + (oc + 1) * 128],
                        ht[kc][:],
                        start=(kc == 0),
                        stop=(kc == MC1 - 1),
                    )
                    if kc == MC1 - 1:
                        # split the bias-adds across DVE and ScalarE (GpSimd
                        # cannot read PSUM) so the last chunk's tail is two
                        # parallel pairs, not four serialized ops.
                        if oc % 2 == 0:
                            nc.vector.tensor_scalar_add(
                                yt[:, oc * ns : (oc + 1) * ns],
                                ps2[oc][:],
                                bbt[:, MC1 + oc : MC1 + oc + 1],
                            )
                        else:
                            nc.scalar.activation(
                                yt[:, oc * ns : (oc + 1) * ns],
                                ps2[oc][:],
                                mybir.ActivationFunctionType.Identity,
                                bias=bbt[:, MC1 + oc : MC1 + oc + 1],
                            )
            if ci == len(chunks) - 1:
                # last chunk is the exec-time tail: store the first half as
                # soon as its bias-adds land instead of waiting for all 4.
                nc.sync.dma_start(
                    yC[:, MC2 * n0 : MC2 * n0 + 2 * ns], yt[:, : 2 * ns]
                )
                nc.sync.dma_start(
                    yC[:, MC2 * n0 + 2 * ns : MC2 * (n0 + ns)], yt[:, 2 * ns :]
                )
            else:
                nc.sync.dma_start(yC[:, MC2 * n0 : MC2 * (n0 + ns)], yt[:])

        emit_mm1(0)
        for ci in range(len(chunks)):
            if ci + 1 < len(chunks):
                emit_mm1(ci + 1)
            emit_mm2(ci)

    nc.compile()
    return nc


def get_program(k_cap):
    if k_cap not in _program_cache:
        _program_cache[k_cap] = _build_program(k_cap)
    return _program_cache[k_cap]


def _softplus(x):
    x = x.astype(np.float64)
    return (np.maximum(x, 0.0) + np.log1p(np.exp(-np.abs(x)))).astype(np.float32)


def _pack_chunks(arrT, chunks, nblk):
    """[nblk*128, k_cap] -> [128, nblk*k_cap] chunk-packed."""
    k_cap = arrT.shape[1]
    out = np.empty((128, nblk * k_cap), dtype=arrT.dtype)
    for n0, ns in chunks:
        out[:, nblk * n0 : nblk * (n0 + ns)] = (
            arrT[:, n0 : n0 + ns].reshape(nblk, 128, ns).transpose(1, 0, 2).reshape(128, nblk * ns)
        )
    return out


def _unpack_chunks(arrC, chunks, nblk):
    """[128, nblk*k_cap] chunk-packed -> [nblk*128, k_cap]."""
    k_cap = arrC.shape[1] // nblk
    out = np.empty((nblk * 128, k_cap), dtype=arrC.dtype)
    for n0, ns in chunks:
        out[:, n0 : n0 + ns] = (
            arrC[:, nblk * n0 : nblk * (n0 + ns)].reshape(128, nblk, ns).transpose(1, 0, 2).reshape(nblk * 128, ns)
        )
    return out


def kernel(epsilon, comp_idx, mu, rho, W1, b1, W2, b2, _trace=False):
    epsilon = np.asarray(epsilon, dtype=np.float32)
    comp_idx = np.asarray(comp_idx, dtype=np.int32)
    mu = np.asarray(mu, dtype=np.float32)
    rho = np.asarray(rho, dtype=np.float32)
    W1 = np.asarray(W1, dtype=np.float32)
    b1 = np.asarray(b1, dtype=np.float32)
    W2 = np.asarray(W2, dtype=np.float32)
    b2 = np.asarray(b2, dtype=np.float32)

    n = epsilon.shape[0]
    sigma = _softplus(rho)  # [C]

    sels = [np.nonzero(comp_idx == c)[0] for c in range(NB_COMP)]
    counts = [len(s) for s in sels]
    k_cap = max(64, -(-max(counts) // 8) * 8)

    nc = get_program(k_cap)
    chunks = _make_chunks(k_cap)

    in_maps = []
    for c in range(NB_COMP):
        sel = sels[c]
        epsT = np.zeros((LAT_DIM, k_cap), dtype=NP_BF16)
        if len(sel):
            epsT[:, : len(sel)] = epsilon[sel].T
        w1p = (W1[c] * sigma[c]).astype(NP_BF16)
        b1p = (
            b1[c].astype(np.float64) + mu[c].astype(np.float64) @ W1[c].astype(np.float64)
        ).astype(np.float32)
        bb = np.empty((128, MC1 + MC2), dtype=np.float32)
        bb[:, :MC1] = b1p.reshape(MC1, 128).T
        bb[:, MC1:] = b2[c].reshape(MC2, 128).T
        w2p = np.ascontiguousarray(
            W2[c].astype(NP_BF16).reshape(MC1, 128, OUT_DIM).transpose(1, 0, 2).reshape(128, MC1 * OUT_DIM)
        )
        in_maps.append(
            {
                "epsC": _pack_chunks(epsT, chunks, KC1),
                "w1": np.ascontiguousarray(w1p),
                "w2": w2p,
                "bb": bb,
            }
        )

    res = run_bass_kernel_spmd(
        nc,
        in_maps,
        core_ids=list(range(N_CORES)),
        trace=_trace,
        trace_cores=list(range(N_CORES)) if _trace else None,
    )

    out = np.zeros((n, OUT_DIM), dtype=np.float32)
    for c in range(NB_COMP):
        sel = sels[c]
        if len(sel):
            yT = _unpack_chunks(res.results[c]["yC"], chunks, MC2)
            out[sel] = yT[:, : len(sel)].T.astype(np.float32)
    if _trace:
        return out, res
    return out


# revision 24
# speedup vs baseline: 1.0390x; 1.0021x over previous
"""GNN ensemble MoE-routing kernel for Trainium2 (8 NeuronCores).

Reference computes all 8 expert MLPs for every sample then selects one
(8x wasted FLOPs). This kernel routes on the host instead: samples are
gathered per expert, and core c runs ONLY expert c's MLP over the
samples routed to it (expert-parallel sharding).

Math folding (exact):
  lat = eps*sigma_c + mu_c  =>  lat @ W1_c = eps @ (sigma_c*W1_c) + mu_c@W1_c
so the device computes  sigmoid(eps @ W1p + b1p) @ W2 + b2  with
  W1p = sigma_c * W1_c,  b1p = b1_c + mu_c @ W1_c  (folded on host).

Device layout: features on SBUF partitions, samples on the free axis.
  epsT [512, K]  ->  H^T [1024, K]  ->  Y^T [512, K]
All matmul operands are bf16 (1 cycle/row at any free-dim size, vs
fp32r's >=256 restriction; also halves DMA bytes and LDWEIGHTS time).
Measured bf16 end-to-end rms rel err ~3e-3, well under the 2e-2 gate.

DMA descriptor writes cost ~0.6us each (serialized per HWDGE ring), so
transfers are packed: the 4 partition-block columns of each input chunk
ship as ONE descriptor (host pre-packs them side-by-side per chunk),
each chunk's 4 output blocks ship as one descriptor, w2 is one packed
[128, 4096] tile (2 transfers), b1+b2 share one [128, 12] f32 tile.
Weights go on the Sync HWDGE ring, activations on the Scalar ring, so
the first matmul's two dependencies land in parallel; later chunks'
loads are deferred into the compute stream so the bandwidth-limited
early drain (~220 GB/s/core, chip-HBM-bound with 8 cores loading
weights at once) serves the weights first. Dummy matmuls on a memset
tile warm the PE out of its cold p-state while the first loads are in
flight.

Chunks are [512, ..., rem-256, 256]: big chunks amortize LDWEIGHTS, the
small last chunk shortens the post-matmul tail (bias-add + final
store, split in two so the first half stores early). k_cap rounds the
max expert count to 8, minimizing padded columns.
"""

from contextlib import ExitStack

import numpy as np
import ml_dtypes

import concourse.bass as bass
import concourse.tile as tile
from concourse import bacc, mybir
from concourse.bass_utils import run_bass_kernel_spmd

NB_COMP = 8
LAT_DIM = 512
NB_NEUR = 1024
OUT_DIM = 512
N_CORES = 8
KC1 = LAT_DIM // 128   # 4 input row-blocks
MC1 = NB_NEUR // 128   # 8 hidden row-blocks
MC2 = OUT_DIM // 128   # 4 output row-blocks

F32 = mybir.dt.float32
BF16 = mybir.dt.bfloat16
NP_BF16 = ml_dtypes.bfloat16
SIG = mybir.ActivationFunctionType.Sigmoid

_program_cache = {}


def _make_chunks(k_cap):
    # 512-wide chunks; the remainder splits so the LAST chunk is small
    # (<=256): it defines the post-matmul tail (bias-add + final store).
    chunks = []
    n0 = 0
    while k_cap - n0 > 576:
        chunks.append((n0, 512))
        n0 += 512
    rem = k_cap - n0
    if rem > 256:
        chunks.append((n0, rem - 256))
        n0 += rem - 256
        rem = 256
    if rem:
        chunks.append((n0, rem))
    return chunks


def _build_program(k_cap):
    """One-expert MLP over k_cap samples; same program runs SPMD on all 8 cores."""
    chunks = _make_chunks(k_cap)

    nc = bacc.Bacc(
        "TRN2",
        target_bir_lowering=False,
        debug=False,
        enable_asserts=False,
        num_devices=N_CORES,
    )
    # chunk-packed layouts: chunk ci at columns [KC1*n0, KC1*(n0+ns)) with
    # its KC1 (resp. MC2) partition-blocks side by side.
    epsC = nc.dram_tensor("epsC", [128, KC1 * k_cap], BF16, kind="ExternalInput").ap()
    w1 = nc.dram_tensor("w1", [LAT_DIM, NB_NEUR], BF16, kind="ExternalInput").ap()
    w2 = nc.dram_tensor("w2", [128, MC1 * OUT_DIM], BF16, kind="ExternalInput").ap()
    bb = nc.dram_tensor("bb", [128, MC1 + MC2], F32, kind="ExternalInput").ap()
    yC = nc.dram_tensor("yC", [128, MC2 * k_cap], BF16, kind="ExternalOutput").ap()

    with tile.TileContext(nc) as tc, ExitStack() as ctx:
        wpool = ctx.enter_context(tc.tile_pool(name="weights", bufs=1))
        xpool = ctx.enter_context(tc.tile_pool(name="x", bufs=len(chunks)))
        hpool = ctx.enter_context(tc.tile_pool(name="h", bufs=2))
        ypool = ctx.enter_context(tc.tile_pool(name="y", bufs=2))
        # mm1 keeps 8 accumulators live (kc-outer order), mm2 4, cycling
        # through the same 8 PSUM banks.
        pspool = ctx.enter_context(tc.tile_pool(name="ps", bufs=8, space="PSUM"))

        # Warm-up: the PE runs at a reduced p-state until ~3us of
        # continuous execution. Run dummy matmuls on a memset tile while
        # the first weight/input DMAs are in flight so real matmuls start
        # at full clock.
        warm = wpool.tile([128, 640], BF16, tag="warm")
        nc.gpsimd.memset(warm[:], 0)
        ps_w = pspool.tile([128, 512], F32, tag="ps", name="ps_warm")
        for _ in range(4):
            nc.tensor.matmul(
                ps_w[:], warm[:, 0:128], warm[:, 128:640], start=True, stop=True
            )

        # Two physical HWDGE rings exist (Sync and Scalar), FIFO per issuing
        # engine, ~0.6us per descriptor write. Split the input stream across
        # them: weights on Sync, activations (and the tiny bias pack) on
        # Scalar, so the first matmul's dependencies (w1_0 on one ring, the
        # first x block on the other) land in parallel. w2 is split in two
        # so mm2's kc<4 half can arrive before its first use.
        w1t = []
        t = wpool.tile([128, NB_NEUR], BF16, tag="w1_0", name="w1_0")
        # only the mc=0 slice of w1_0 gates the first real matmul's weight
        # load; ship it first as its own tiny transfer.
        nc.sync.dma_start(t[:, 0:128], w1[0:128, 0:128])
        nc.sync.dma_start(t[:, 128:], w1[0:128, 128:])
        w1t.append(t)
        n0_0, ns_0 = chunks[0]
        # split the first chunk's load: only the kc=0 block gates the first
        # real matmul, the rest follows while kc=0 computes.
        x0 = xpool.tile([128, KC1 * ns_0], BF16, tag="x", name="x0")
        nc.scalar.dma_start(x0[:, :ns_0], epsC[:, 0:ns_0])
        for kc in range(1, KC1):
            t = wpool.tile([128, NB_NEUR], BF16, tag=f"w1_{kc}", name=f"w1_{kc}")
            nc.sync.dma_start(t[:], w1[kc * 128 : (kc + 1) * 128, :])
            w1t.append(t)
        nc.scalar.dma_start(x0[:, ns_0:], epsC[:, ns_0 : KC1 * ns_0])
        bbt = wpool.tile([128, MC1 + MC2], F32, tag="bb")
        nc.scalar.dma_start(bbt[:], bb[:])
        w2t = wpool.tile([128, MC1 * OUT_DIM], BF16, tag="w2")
        half = MC1 * OUT_DIM // 2
        nc.sync.dma_start(w2t[:, :half], w2[:, :half])
        nc.sync.dma_start(w2t[:, half:], w2[:, half:])
        # Software pipeline: mm1(ci+1) runs BEFORE mm2(ci), so every w2 /
        # x-chunk deadline moves one whole mm1 stage (~7us) later. The head
        # DMA stream is bandwidth-limited (~220 GB/s/core, chip-HBM-bound);
        # without the lookahead, chunk0's mm2 start sits exactly at w2's
        # arrival, so any per-core DMA variance stalls the PE. x1 loads
        # upfront (its deadline is mm1(1), right after mm1(0)); later
        # chunks' loads are emitted at the previous chunk's first
        # activation (Scalar program order).
        xts = [x0]
        for n0, ns in chunks[1:]:
            xts.append(xpool.tile([128, KC1 * ns], BF16, tag="x", name=f"x_{n0}"))

        def load_x(ci):
            n0, ns = chunks[ci]
            nc.scalar.dma_start(xts[ci][:], epsC[:, KC1 * n0 : KC1 * (n0 + ns)])

        if len(chunks) > 1:
            load_x(1)

        hts = {}

        def emit_mm1(ci):
            n0, ns = chunks[ci]
            xt = xts[ci]
            ht = []
            ps1 = [
                pspool.tile([128, ns], F32, tag="ps", name=f"ps1_{ci}_{i}")
                for i in range(MC1)
            ]
            for kc in range(KC1):
                for mc in range(MC1):
                    nc.tensor.matmul(
                        ps1[mc][:],
                        w1t[kc][:, mc * 128 : (mc + 1) * 128],
                        xt[:, kc * ns : (kc + 1) * ns],
                        start=(kc == 0),
                        stop=(kc == KC1 - 1),
                    )
                    if kc == KC1 - 1:
                        h = hpool.tile([128, ns], BF16, tag=f"h{mc}")
                        nc.scalar.activation(
                            h[:], ps1[mc][:], SIG, bias=bbt[:, mc : mc + 1]
                        )
                        ht.append(h)
                        if mc == 0 and ci + 2 < len(chunks):
                            load_x(ci + 2)
            hts[ci] = ht

        def emit_mm2(ci):
            n0, ns = chunks[ci]
            ht = hts.pop(ci)
            yt = ypool.tile([128, MC2 * ns], BF16, tag="y", name=f"y_{ci}")
            ps2 = [
                pspool.tile([128, ns], F32, tag="ps", name=f"ps2_{ci}_{i}")
                for i in range(MC2)
            ]
            for kc in range(MC1):
                for oc in range(MC2):
                    nc.tensor.matmul(
                        ps2[oc][:],
                        w2t[:, kc * OUT_DIM + oc * 128 : kc * OUT_DIM + (oc + 1) * 128],
                        ht[kc][:],
                        start=(kc == 0),
                        stop=(kc == MC1 - 1),
                    )
                    if kc == MC1 - 1:
                        # split the bias-adds across DVE and ScalarE (GpSimd
                        # cannot read PSUM) so the last chunk's tail is two
                        # parallel pairs, not four serialized ops.
                        if oc % 2 == 0:
                            nc.vector.tensor_scalar_add(
                                yt[:, oc * ns : (oc + 1) * ns],
                                ps2[oc][:],
                                bbt[:, MC1 + oc : MC1 + oc + 1],
                            )
                        else:
                            nc.scalar.activation(
                                yt[:, oc * ns : (oc + 1) * ns],
                                ps2[oc][:],
                                mybir.ActivationFunctionType.Identity,
                                bias=bbt[:, MC1 + oc : MC1 + oc + 1],
                            )
            if ci == len(chunks) - 1:
                # last chunk is the exec-time tail: store the first half as
                # soon as its bias-adds land instead of waiting for all 4.
                nc.sync.dma_start(
                    yC[:, MC2 * n0 : MC2 * n0 + 2 * ns], yt[:, : 2 * ns]
                )
                nc.sync.dma_start(
                    yC[:, MC2 * n0 + 2 * ns : MC2 * (n0 + ns)], yt[:, 2 * ns :]
                )
            else:
                nc.sync.dma_start(yC[:, MC2 * n0 : MC2 * (n0 + ns)], yt[:])

        emit_mm1(0)
        for ci in range(len(chunks)):
            if ci + 1 < len(chunks):
                emit_mm1(ci + 1)
            emit_mm2(ci)

    nc.compile()
    return nc


def get_program(k_cap):
    if k_cap not in _program_cache:
        _program_cache[k_cap] = _build_program(k_cap)
    return _program_cache[k_cap]


def _softplus(x):
    x = x.astype(np.float64)
    return (np.maximum(x, 0.0) + np.log1p(np.exp(-np.abs(x)))).astype(np.float32)


def _pack_chunks(arrT, chunks, nblk):
    """[nblk*128, k_cap] -> [128, nblk*k_cap] chunk-packed."""
    k_cap = arrT.shape[1]
    out = np.empty((128, nblk * k_cap), dtype=arrT.dtype)
    for n0, ns in chunks:
        out[:, nblk * n0 : nblk * (n0 + ns)] = (
            arrT[:, n0 : n0 + ns].reshape(nblk, 128, ns).transpose(1, 0, 2).reshape(128, nblk * ns)
        )
    return out


def _unpack_chunks(arrC, chunks, nblk):
    """[128, nblk*k_cap] chunk-packed -> [nblk*128, k_cap]."""
    k_cap = arrC.shape[1] // nblk
    out = np.empty((nblk * 128, k_cap), dtype=arrC.dtype)
    for n0, ns in chunks:
        out[:, n0 : n0 + ns] = (
            arrC[:, nblk * n0 : nblk * (n0 + ns)].reshape(128, nblk, ns).transpose(1, 0, 2).reshape(nblk * 128, ns)
        )
    return out


def kernel(epsilon, comp_idx, mu, rho, W1, b1, W2, b2, _trace=False):
    epsilon = np.asarray(epsilon, dtype=np.float32)
    comp_idx = np.asarray(comp_idx, dtype=np.int32)
    mu = np.asarray(mu, dtype=np.float32)
    rho = np.asarray(rho, dtype=np.float32)
    W1 = np.asarray(W1, dtype=np.float32)
    b1 = np.asarray(b1, dtype=np.float32)
    W2 = np.asarray(W2, dtype=np.float32)
    b2 = np.asarray(b2, dtype=np.float32)

    n = epsilon.shape[0]
    sigma = _softplus(rho)  # [C]

    sels = [np.nonzero(comp_idx == c)[0] for c in range(NB_COMP)]
    counts = [len(s) for s in sels]
    k_cap = max(64, -(-max(counts) // 8) * 8)

    nc = get_program(k_cap)
    chunks = _make_chunks(k_cap)

    in_maps = []
    for c in range(NB_COMP):
        sel = sels[c]
        epsT = np.zeros((LAT_DIM, k_cap), dtype=NP_BF16)
        if len(sel):
            epsT[:, : len(sel)] = epsilon[sel].T
        w1p = (W1[c] * sigma[c]).astype(NP_BF16)
        b1p = (
            b1[c].astype(np.float64) + mu[c].astype(np.float64) @ W1[c].astype(np.float64)
        ).astype(np.float32)
        bb = np.empty((128, MC1 + MC2), dtype=np.float32)
        bb[:, :MC1] = b1p.reshape(MC1, 128).T
        bb[:, MC1:] = b2[c].reshape(MC2, 128).T
        w2p = np.ascontiguousarray(
            W2[c].astype(NP_BF16).reshape(MC1, 128, OUT_DIM).transpose(1, 0, 2).reshape(128, MC1 * OUT_DIM)
        )
        in_maps.append(
            {
                "epsC": _pack_chunks(epsT, chunks, KC1),
                "w1": np.ascontiguousarray(w1p),
                "w2": w2p,
                "bb": bb,
            }
        )

    res = run_bass_kernel_spmd(
        nc,
        in_maps,
        core_ids=list(range(N_CORES)),
        trace=_trace,
        trace_cores=list(range(N_CORES)) if _trace else None,
    )

    out = np.zeros((n, OUT_DIM), dtype=np.float32)
    for c in range(NB_COMP):
        sel = sels[c]
        if len(sel):
            yT = _unpack_chunks(res.results[c]["yC"], chunks, MC2)
            out[sel] = yT[:, : len(sel)].T.astype(np.float32)
    if _trace:
        return out, res
    return out
